# revision 3
# baseline (speedup 1.0000x reference)
"""DeepSeekMoE layer on 8 Trainium2 NeuronCores.

Strategy (expert-parallel, matching the sharding hint):
  - Host computes the (tiny) gate: softmax(x @ gate_w + gate_b), top-2
    routing, and per-expert token gather with capacity padding.  This is
    the control plane (<1% of FLOPs); all heavy matmuls run on device.
  - Each of the 8 cores owns E/8 = 4 routed experts: it receives the
    gathered tokens for those experts (transposed, bf16), the expert
    weights (bf16), and the per-(token,expert) combine weights.
  - The 2 shared experts are data-parallel over tokens: core c processes
    tokens [c*512, (c+1)*512) for both shared experts.
  - Device per expert: hT = w1^T @ xT (K=D in PSUM) -> Gelu(+b1) on ACT
    -> combine-weight multiply on DVE -> yT = w2^T @ gT (K=FD) -> out.
    Shared experts fuse their second matmul into one K=2*FD accumulation.
  - Host scatter-adds routed expert outputs + shared outputs back into
    token order (each token appears in exactly 2 routed lists + 1 shared
    list, so a fp32 sum reproduces the reference combine).
All matmul inputs are bf16 (PSUM accumulates fp32); biases are applied
in fp32 (b1 via the ACT bias port; b2/gate_b host-side, and they are
zero-guarded so the common all-zero case costs nothing).
"""

import os
import sys
import types

import numpy as np

# ---------------------------------------------------------------------------
# Optional NTFF trace support under axon: concourse's trace path imports
# antenv.axon_hooks, which this image lacks; shim it with the boot helper.
# ---------------------------------------------------------------------------
def _install_trace_shim():
    try:
        if "antenv.axon_hooks" in sys.modules:
            return
        from trn_agent_boot.trn_boot import _ntff_profile_via_ctypes

        hook = _ntff_profile_via_ctypes("/opt/axon/libaxon_pjrt.so")
        mod = types.ModuleType("antenv.axon_hooks")
        mod.get_axon_ntff_profile_hook = lambda: hook
        mod.set_axon_ntff_profile_hook = lambda h: None
        sys.modules["antenv.axon_hooks"] = mod
    except Exception:
        pass


_install_trace_shim()

import ml_dtypes  # noqa: E402

import concourse.bacc as bacc  # noqa: E402
import concourse.bass as bass  # noqa: E402
import concourse.mybir as mybir  # noqa: E402
import concourse.tile as tile  # noqa: E402
import concourse.bass_utils as bass_utils  # noqa: E402
from concourse.bass import ts  # noqa: E402
from concourse.bass_utils import run_bass_kernel_spmd  # noqa: E402

try:  # zero-egress sandbox: skip artifact uploads during tracing
    bass_utils.upload_artifacts = lambda tmpdir: tmpdir
except Exception:
    pass

# Problem shapes (nn_DeepSeekMoE): x [B,T,D]; E routed experts (top-K),
# S shared experts, ffn dim FD.
B, T, D = 2, 2048, 5120
FD, E, S, K = 384, 32, 2, 2
N = B * T                     # 4096 tokens
P = 128
NCORES = 8
EPC = E // NCORES             # 4 routed experts per core
NS = N // NCORES              # 512 shared-expert tokens per core
KD = D // P                   # 40 K-subtiles over D
KF = FD // P                  # 3 K-subtiles over FD
MD = D // P                   # 40 output subtiles over D
MD_CHUNK = 4                  # md subtiles per streamed w2 chunk (512 cols)

BF16 = ml_dtypes.bfloat16

LAST_EXEC_NS = None
LAST_MEAN_EXEC_NS = None
LAST_TRACE = None


def _build_nc(C: int, out_dt=mybir.dt.float32):
    """Build the SPMD per-core Bass program. C = per-expert token capacity."""
    f32 = mybir.dt.float32
    bf16 = mybir.dt.bfloat16
    nc = bacc.Bacc(None, target_bir_lowering=False)

    xeT = nc.dram_tensor("xeT", (EPC, D, C), bf16, kind="ExternalInput")
    wgt = nc.dram_tensor("wgt", (EPC, P, C), f32, kind="ExternalInput")
    rw1 = nc.dram_tensor("rw1", (EPC, D, FD), bf16, kind="ExternalInput")
    rb1 = nc.dram_tensor("rb1", (EPC, FD), f32, kind="ExternalInput")
    rw2 = nc.dram_tensor("rw2", (EPC, FD, D), bf16, kind="ExternalInput")
    xsT = nc.dram_tensor("xsT", (D, NS), bf16, kind="ExternalInput")
    sw1 = nc.dram_tensor("sw1", (S, D, FD), bf16, kind="ExternalInput")
    sb1 = nc.dram_tensor("sb1", (S, FD), f32, kind="ExternalInput")
    sw2 = nc.dram_tensor("sw2", (S, FD, D), bf16, kind="ExternalInput")
    yeT = nc.dram_tensor("yeT", (EPC, D, C), out_dt, kind="ExternalOutput")
    ysT = nc.dram_tensor("ysT", (D, NS), out_dt, kind="ExternalOutput")

    gelu = mybir.ActivationFunctionType.Gelu

    with tile.TileContext(nc) as tc:
        with (
            tc.tile_pool(name="pw1", bufs=2) as pw1,
            tc.tile_pool(name="pxe", bufs=2) as pxe,
            tc.tile_pool(name="pw2", bufs=4) as pw2,
            tc.tile_pool(name="pg", bufs=3) as pg,
            tc.tile_pool(name="pwg", bufs=2) as pwg,
            tc.tile_pool(name="pyt", bufs=6) as pyt,
            tc.tile_pool(name="pb", bufs=1) as pb,
            tc.tile_pool(name="pps1", bufs=2, space="PSUM") as pps1,
            tc.tile_pool(name="pps2", bufs=4, space="PSUM") as pps2,
        ):
            # biases -> [P, n_subtiles] with the FD subtile index on free dim
            rb1_sb = pb.tile([P, EPC * KF], f32, tag="rb1")
            for e in range(EPC):
                nc.sync.dma_start(
                    rb1_sb[:, e * KF : (e + 1) * KF],
                    rb1[:][e].rearrange("(o p) -> p o", p=P),
                )
            sb1_sb = pb.tile([P, S * KF], f32, tag="sb1")
            for s in range(S):
                nc.sync.dma_start(
                    sb1_sb[:, s * KF : (s + 1) * KF],
                    sb1[:][s].rearrange("(o p) -> p o", p=P),
                )

            def ffn_mm1(w1_ap, x_tile, g_tile, bias_sb, bias_off, ntok, wgt_tile):
                """g = [gelu(w1^T x + b1) * wgt]^T, FD-major: [P, KF, ntok]."""
                w1t = pw1.tile([P, KD, FD], bf16, tag="w1")
                nc.sync.dma_start(w1t, w1_ap.rearrange("(o p) f -> p o f", p=P))
                for mi in range(KF):
                    ph = pps1.tile([P, 512], f32, tag="ph", name="ph")[:, :ntok]
                    for kd in range(KD):
                        nc.tensor.matmul(
                            ph,
                            w1t[:, kd, ts(mi, P)],
                            x_tile[:, kd, :],
                            start=(kd == 0),
                            stop=(kd == KD - 1),
                        )
                    nc.scalar.activation(
                        g_tile[:, mi, :],
                        ph,
                        gelu,
                        bias=bias_sb[:, bias_off + mi : bias_off + mi + 1],
                    )
                    if wgt_tile is not None:
                        nc.vector.tensor_mul(
                            out=g_tile[:, mi, :],
                            in0=g_tile[:, mi, :],
                            in1=wgt_tile,
                        )

            # ---------------- routed experts ----------------
            for e in range(EPC):
                xet = pxe.tile([P, KD, C], bf16, tag="xe")
                nc.sync.dma_start(xet, xeT[:][e].rearrange("(o p) c -> p o c", p=P))
                wgt_t = pwg.tile([P, C], f32, tag="wg")
                nc.sync.dma_start(wgt_t, wgt[:][e])
                gt = pg.tile([P, KF, C], bf16, tag="g")
                ffn_mm1(rw1[:][e], xet, gt, rb1_sb, e * KF, C, wgt_t)

                w2_ap = rw2[:][e].rearrange("(o p) d -> p o d", p=P)
                for mdc in range(MD // MD_CHUNK):
                    w2t = pw2.tile([P, KF, MD_CHUNK * P], bf16, tag="w2")
                    nc.sync.dma_start(
                        w2t, w2_ap[:, :, ts(mdc, MD_CHUNK * P)]
                    )
                    for mdi in range(MD_CHUNK):
                        md = mdc * MD_CHUNK + mdi
                        py = pps2.tile([P, 512], f32, tag="py", name="py")[:, :C]
                        for kf in range(KF):
                            nc.tensor.matmul(
                                py,
                                w2t[:, kf, ts(mdi, P)],
                                gt[:, kf, :],
                                start=(kf == 0),
                                stop=(kf == KF - 1),
                            )
                        yt = pyt.tile([P, 512], out_dt, tag="yt", name="yt")[:, :C]
                        nc.vector.tensor_copy(out=yt, in_=py)
                        nc.sync.dma_start(yeT[:][e, ts(md, P), :], yt)

            # ---------------- shared experts (token-parallel) ----------------
            xst = pxe.tile([P, KD, NS], bf16, tag="xe")
            nc.sync.dma_start(xst, xsT[:].rearrange("(o p) c -> p o c", p=P))
            gs = []
            for s in range(S):
                g_s = pg.tile([P, KF, NS], bf16, tag="g")
                ffn_mm1(sw1[:][s], xst, g_s, sb1_sb, s * KF, NS, None)
                gs.append(g_s)

            sw2_aps = [
                sw2[:][s].rearrange("(o p) d -> p o d", p=P) for s in range(S)
            ]
            for mdc in range(MD // MD_CHUNK):
                w2ts = []
                for s in range(S):
                    w2t = pw2.tile([P, KF, MD_CHUNK * P], bf16, tag="w2")
                    nc.sync.dma_start(w2t, sw2_aps[s][:, :, ts(mdc, MD_CHUNK * P)])
                    w2ts.append(w2t)
                for mdi in range(MD_CHUNK):
                    md = mdc * MD_CHUNK + mdi
                    py = pps2.tile([P, 512], f32, tag="py", name="py")[:, :NS]
                    for s in range(S):
                        for kf in range(KF):
                            nc.tensor.matmul(
                                py,
                                w2ts[s][:, kf, ts(mdi, P)],
                                gs[s][:, kf, :],
                                start=(s == 0 and kf == 0),
                                stop=(s == S - 1 and kf == KF - 1),
                            )
                    yt = pyt.tile([P, 512], out_dt, tag="yt", name="yt")[:, :NS]
                    nc.vector.tensor_copy(out=yt, in_=py)
                    nc.sync.dma_start(ysT[:][ts(md, P), :], yt)

    nc.compile()
    return nc


def kernel(x, shared_w1, shared_b1, shared_w2, shared_b2,
           routed_w1, routed_b1, routed_w2, routed_b2, gate_w, gate_b):
    global LAST_EXEC_NS, LAST_MEAN_EXEC_NS, LAST_TRACE

    x = np.asarray(x, np.float32)
    x2d = np.ascontiguousarray(x.reshape(N, D))

    # ---- gating / routing (control plane) ----
    logits = x2d @ np.asarray(gate_w, np.float32) + np.asarray(gate_b, np.float32)
    logits -= logits.max(axis=-1, keepdims=True)
    probs = np.exp(logits)
    probs /= probs.sum(axis=-1, keepdims=True)                  # [N, E]
    top2 = np.argpartition(-probs, K - 1, axis=-1)[:, :K]       # [N, K]
    sel = np.zeros((N, E), np.bool_)
    sel[np.arange(N)[:, None], top2] = True

    idx_per_e = [np.nonzero(sel[:, e])[0] for e in range(E)]
    counts = np.array([len(i) for i in idx_per_e])
    C = max(384, int(np.ceil(counts.max() / P)) * P)

    x_bf = x2d.astype(BF16)

    in_maps = []
    for c in range(NCORES):
        xeT = np.zeros((EPC, D, C), BF16)
        wgt = np.zeros((EPC, P, C), np.float32)
        for j in range(EPC):
            e = c * EPC + j
            idx = idx_per_e[e]
            xeT[j, :, : len(idx)] = x_bf[idx].T
            wgt[j, :, : len(idx)] = probs[idx, e][None, :]
        in_maps.append(
            {
                "xeT": xeT,
                "wgt": wgt,
                "rw1": np.ascontiguousarray(
                    np.asarray(routed_w1)[c * EPC : (c + 1) * EPC]
                ).astype(BF16),
                "rb1": np.ascontiguousarray(
                    np.asarray(routed_b1, np.float32)[c * EPC : (c + 1) * EPC]
                ),
                "rw2": np.ascontiguousarray(
                    np.asarray(routed_w2)[c * EPC : (c + 1) * EPC]
                ).astype(BF16),
                "xsT": np.ascontiguousarray(x_bf[c * NS : (c + 1) * NS].T),
                "sw1": np.asarray(shared_w1).astype(BF16),
                "sb1": np.asarray(shared_b1, np.float32),
                "sw2": np.asarray(shared_w2).astype(BF16),
            }
        )

    nc = _build_nc(C)
    trace = bool(int(os.environ.get("MOE_TRACE", "0")))
    res = run_bass_kernel_spmd(
        nc, in_maps, core_ids=list(range(NCORES)), trace=trace
    )
    LAST_EXEC_NS = res.exec_time_ns
    LAST_MEAN_EXEC_NS = res.mean_exec_time_ns
    LAST_TRACE = res.instructions_and_trace[1] if res.instructions_and_trace else None

    # ---- combine (un-shard) ----
    out = np.zeros((N, D), np.float32)
    for c in range(NCORES):
        out[c * NS : (c + 1) * NS] = np.asarray(res.results[c]["ysT"], np.float32).T
    for c in range(NCORES):
        yeT = res.results[c]["yeT"]
        for j in range(EPC):
            e = c * EPC + j
            idx = idx_per_e[e]
            out[idx] += np.asarray(yeT[j, :, : len(idx)], np.float32).T

    # host-side bias terms (zero in the shipped init; guarded for generality)
    shared_b2 = np.asarray(shared_b2, np.float32)
    if np.any(shared_b2):
        out += shared_b2.sum(0)[None, :]
    routed_b2 = np.asarray(routed_b2, np.float32)
    if np.any(routed_b2):
        w_full = np.where(sel, probs, 0.0).astype(np.float32)
        out += w_full @ routed_b2

    return out.reshape(B, T, D)


# revision 4
# speedup vs baseline: 1.0652x; 1.0652x over previous
"""DeepSeekMoE layer on 8 Trainium2 NeuronCores.

Strategy (expert-parallel, matching the sharding hint):
  - Host computes the (tiny) gate: softmax(x @ gate_w + gate_b), top-2
    routing, and per-expert token gather with capacity padding.  This is
    the control plane (<1% of FLOPs); all heavy matmuls run on device.
  - Each of the 8 cores owns E/8 = 4 routed experts: it receives the
    gathered tokens for those experts (transposed, bf16), the expert
    weights (bf16), and the per-(token,expert) combine weights.
  - The 2 shared experts are data-parallel over tokens: core c processes
    tokens [c*512, (c+1)*512) for both shared experts.
  - Device per expert: hT = w1^T @ xT (K=D in PSUM) -> Gelu(+b1) on ACT
    -> combine-weight multiply on DVE -> yT = w2^T @ gT (K=FD) -> out.
    Shared experts fuse their second matmul into one K=2*FD accumulation.
  - Host scatter-adds routed expert outputs + shared outputs back into
    token order (each token appears in exactly 2 routed lists + 1 shared
    list, so a fp32 sum reproduces the reference combine).
All matmul inputs are bf16 (PSUM accumulates fp32); biases are applied
in fp32 (b1 via the ACT bias port; b2/gate_b host-side, and they are
zero-guarded so the common all-zero case costs nothing).
"""

import os
import sys
import types

import numpy as np

# ---------------------------------------------------------------------------
# Optional NTFF trace support under axon: concourse's trace path imports
# antenv.axon_hooks, which this image lacks; shim it with the boot helper.
# ---------------------------------------------------------------------------
def _install_trace_shim():
    try:
        if "antenv.axon_hooks" in sys.modules:
            return
        from trn_agent_boot.trn_boot import _ntff_profile_via_ctypes

        hook = _ntff_profile_via_ctypes("/opt/axon/libaxon_pjrt.so")
        mod = types.ModuleType("antenv.axon_hooks")
        mod.get_axon_ntff_profile_hook = lambda: hook
        mod.set_axon_ntff_profile_hook = lambda h: None
        sys.modules["antenv.axon_hooks"] = mod
    except Exception:
        pass


_install_trace_shim()

import ml_dtypes  # noqa: E402

import concourse.bacc as bacc  # noqa: E402
import concourse.bass as bass  # noqa: E402
import concourse.mybir as mybir  # noqa: E402
import concourse.tile as tile  # noqa: E402
import concourse.bass_utils as bass_utils  # noqa: E402
from concourse.bass import ts  # noqa: E402
from concourse.bass_utils import run_bass_kernel_spmd  # noqa: E402

try:  # zero-egress sandbox: skip artifact uploads during tracing
    bass_utils.upload_artifacts = lambda tmpdir: tmpdir
except Exception:
    pass

# Problem shapes (nn_DeepSeekMoE): x [B,T,D]; E routed experts (top-K),
# S shared experts, ffn dim FD.
B, T, D = 2, 2048, 5120
FD, E, S, K = 384, 32, 2, 2
N = B * T                     # 4096 tokens
P = 128
NCORES = 8
EPC = E // NCORES             # 4 routed experts per core
NS = N // NCORES              # 512 shared-expert tokens per core
KD = D // P                   # 40 K-subtiles over D
KF = FD // P                  # 3 K-subtiles over FD
MD = D // P                   # 40 output subtiles over D
MD_CHUNK = 4                  # md subtiles per streamed w2 chunk (512 cols)

BF16 = ml_dtypes.bfloat16

LAST_EXEC_NS = None
LAST_MEAN_EXEC_NS = None
LAST_TRACE = None


def _build_nc(C: int, out_dt=mybir.dt.bfloat16):
    """Build the SPMD per-core Bass program. C = per-expert token capacity."""
    f32 = mybir.dt.float32
    bf16 = mybir.dt.bfloat16
    nc = bacc.Bacc(None, target_bir_lowering=False)

    xeT = nc.dram_tensor("xeT", (EPC, D, C), bf16, kind="ExternalInput")
    wgt = nc.dram_tensor("wgt", (EPC, P, C), f32, kind="ExternalInput")
    rw1 = nc.dram_tensor("rw1", (EPC, D, FD), bf16, kind="ExternalInput")
    rb1 = nc.dram_tensor("rb1", (EPC, FD), f32, kind="ExternalInput")
    rw2 = nc.dram_tensor("rw2", (EPC, FD, D), bf16, kind="ExternalInput")
    xsT = nc.dram_tensor("xsT", (D, NS), bf16, kind="ExternalInput")
    sw1 = nc.dram_tensor("sw1", (S, D, FD), bf16, kind="ExternalInput")
    sb1 = nc.dram_tensor("sb1", (S, FD), f32, kind="ExternalInput")
    sw2 = nc.dram_tensor("sw2", (S, FD, D), bf16, kind="ExternalInput")
    yeT = nc.dram_tensor("yeT", (EPC, D, C), out_dt, kind="ExternalOutput")
    ysT = nc.dram_tensor("ysT", (D, NS), out_dt, kind="ExternalOutput")

    gelu = mybir.ActivationFunctionType.Gelu

    with tile.TileContext(nc) as tc:
        with (
            tc.tile_pool(name="pw1", bufs=2) as pw1,
            tc.tile_pool(name="pxe", bufs=2) as pxe,
            tc.tile_pool(name="pw2", bufs=4) as pw2,
            tc.tile_pool(name="pg", bufs=3) as pg,
            tc.tile_pool(name="pwg", bufs=2) as pwg,
            tc.tile_pool(name="pyt", bufs=6) as pyt,
            tc.tile_pool(name="pb", bufs=1) as pb,
            tc.tile_pool(name="pps1", bufs=2, space="PSUM") as pps1,
            tc.tile_pool(name="pps2", bufs=4, space="PSUM") as pps2,
        ):
            # biases -> [P, n_subtiles] with the FD subtile index on free dim
            rb1_sb = pb.tile([P, EPC * KF], f32, tag="rb1")
            for e in range(EPC):
                nc.sync.dma_start(
                    rb1_sb[:, e * KF : (e + 1) * KF],
                    rb1[:][e].rearrange("(o p) -> p o", p=P),
                )
            sb1_sb = pb.tile([P, S * KF], f32, tag="sb1")
            for s in range(S):
                nc.sync.dma_start(
                    sb1_sb[:, s * KF : (s + 1) * KF],
                    sb1[:][s].rearrange("(o p) -> p o", p=P),
                )

            def ffn_mm1(w1_ap, x_tile, g_tile, bias_sb, bias_off, ntok, wgt_tile):
                """g = [gelu(w1^T x + b1) * wgt]^T, FD-major: [P, KF, ntok]."""
                w1t = pw1.tile([P, KD, FD], bf16, tag="w1")
                w1_r = w1_ap.rearrange("(o p) f -> p o f", p=P)
                nc.sync.dma_start(w1t[:, : KD // 2], w1_r[:, : KD // 2])
                nc.sync.dma_start(w1t[:, KD // 2 :], w1_r[:, KD // 2 :])
                for mi in range(KF):
                    ph = pps1.tile([P, 512], f32, tag="ph", name="ph")[:, :ntok]
                    for kd in range(KD):
                        nc.tensor.matmul(
                            ph,
                            w1t[:, kd, ts(mi, P)],
                            x_tile[:, kd, :],
                            start=(kd == 0),
                            stop=(kd == KD - 1),
                        )
                    nc.scalar.activation(
                        g_tile[:, mi, :],
                        ph,
                        gelu,
                        bias=bias_sb[:, bias_off + mi : bias_off + mi + 1],
                    )
                    if wgt_tile is not None:
                        nc.vector.tensor_mul(
                            out=g_tile[:, mi, :],
                            in0=g_tile[:, mi, :],
                            in1=wgt_tile,
                        )

            # ---------------- routed experts ----------------
            for e in range(EPC):
                xet = pxe.tile([P, KD, C], bf16, tag="xe")
                xe_r = xeT[:][e].rearrange("(o p) c -> p o c", p=P)
                nc.sync.dma_start(xet[:, : KD // 2], xe_r[:, : KD // 2])
                nc.sync.dma_start(xet[:, KD // 2 :], xe_r[:, KD // 2 :])
                wgt_t = pwg.tile([P, C], f32, tag="wg")
                nc.sync.dma_start(wgt_t, wgt[:][e])
                gt = pg.tile([P, KF, C], bf16, tag="g")
                ffn_mm1(rw1[:][e], xet, gt, rb1_sb, e * KF, C, wgt_t)

                w2_ap = rw2[:][e].rearrange("(o p) d -> p o d", p=P)
                for mdc in range(MD // MD_CHUNK):
                    w2t = pw2.tile([P, KF, MD_CHUNK * P], bf16, tag="w2")
                    nc.sync.dma_start(
                        w2t, w2_ap[:, :, ts(mdc, MD_CHUNK * P)]
                    )
                    for mdi in range(MD_CHUNK):
                        md = mdc * MD_CHUNK + mdi
                        py = pps2.tile([P, 512], f32, tag="py", name="py")[:, :C]
                        for kf in range(KF):
                            nc.tensor.matmul(
                                py,
                                w2t[:, kf, ts(mdi, P)],
                                gt[:, kf, :],
                                start=(kf == 0),
                                stop=(kf == KF - 1),
                            )
                        yt = pyt.tile([P, 512], out_dt, tag="yt", name="yt")[:, :C]
                        nc.vector.tensor_copy(out=yt, in_=py)
                        nc.sync.dma_start(yeT[:][e, ts(md, P), :], yt)

            # ---------------- shared experts (token-parallel) ----------------
            xst = pxe.tile([P, KD, NS], bf16, tag="xe")
            xs_r = xsT[:].rearrange("(o p) c -> p o c", p=P)
            nc.sync.dma_start(xst[:, : KD // 2], xs_r[:, : KD // 2])
            nc.sync.dma_start(xst[:, KD // 2 :], xs_r[:, KD // 2 :])
            gs = []
            for s in range(S):
                g_s = pg.tile([P, KF, NS], bf16, tag="g")
                ffn_mm1(sw1[:][s], xst, g_s, sb1_sb, s * KF, NS, None)
                gs.append(g_s)

            sw2_aps = [
                sw2[:][s].rearrange("(o p) d -> p o d", p=P) for s in range(S)
            ]
            for mdc in range(MD // MD_CHUNK):
                w2ts = []
                for s in range(S):
                    w2t = pw2.tile([P, KF, MD_CHUNK * P], bf16, tag="w2")
                    nc.sync.dma_start(w2t, sw2_aps[s][:, :, ts(mdc, MD_CHUNK * P)])
                    w2ts.append(w2t)
                for mdi in range(MD_CHUNK):
                    md = mdc * MD_CHUNK + mdi
                    py = pps2.tile([P, 512], f32, tag="py", name="py")[:, :NS]
                    for s in range(S):
                        for kf in range(KF):
                            nc.tensor.matmul(
                                py,
                                w2ts[s][:, kf, ts(mdi, P)],
                                gs[s][:, kf, :],
                                start=(s == 0 and kf == 0),
                                stop=(s == S - 1 and kf == KF - 1),
                            )
                    yt = pyt.tile([P, 512], out_dt, tag="yt", name="yt")[:, :NS]
                    nc.vector.tensor_copy(out=yt, in_=py)
                    nc.sync.dma_start(ysT[:][ts(md, P), :], yt)

    nc.compile()
    return nc


def kernel(x, shared_w1, shared_b1, shared_w2, shared_b2,
           routed_w1, routed_b1, routed_w2, routed_b2, gate_w, gate_b):
    global LAST_EXEC_NS, LAST_MEAN_EXEC_NS, LAST_TRACE

    x = np.asarray(x, np.float32)
    x2d = np.ascontiguousarray(x.reshape(N, D))

    # ---- gating / routing (control plane) ----
    logits = x2d @ np.asarray(gate_w, np.float32) + np.asarray(gate_b, np.float32)
    logits -= logits.max(axis=-1, keepdims=True)
    probs = np.exp(logits)
    probs /= probs.sum(axis=-1, keepdims=True)                  # [N, E]
    top2 = np.argpartition(-probs, K - 1, axis=-1)[:, :K]       # [N, K]
    sel = np.zeros((N, E), np.bool_)
    sel[np.arange(N)[:, None], top2] = True

    idx_per_e = [np.nonzero(sel[:, e])[0] for e in range(E)]
    counts = np.array([len(i) for i in idx_per_e])
    C = max(128, int(np.ceil(counts.max() / 32)) * 32)

    x_bf = x2d.astype(BF16)

    in_maps = []
    for c in range(NCORES):
        xeT = np.zeros((EPC, D, C), BF16)
        wgt = np.zeros((EPC, P, C), np.float32)
        for j in range(EPC):
            e = c * EPC + j
            idx = idx_per_e[e]
            xeT[j, :, : len(idx)] = x_bf[idx].T
            wgt[j, :, : len(idx)] = probs[idx, e][None, :]
        in_maps.append(
            {
                "xeT": xeT,
                "wgt": wgt,
                "rw1": np.ascontiguousarray(
                    np.asarray(routed_w1)[c * EPC : (c + 1) * EPC]
                ).astype(BF16),
                "rb1": np.ascontiguousarray(
                    np.asarray(routed_b1, np.float32)[c * EPC : (c + 1) * EPC]
                ),
                "rw2": np.ascontiguousarray(
                    np.asarray(routed_w2)[c * EPC : (c + 1) * EPC]
                ).astype(BF16),
                "xsT": np.ascontiguousarray(x_bf[c * NS : (c + 1) * NS].T),
                "sw1": np.asarray(shared_w1).astype(BF16),
                "sb1": np.asarray(shared_b1, np.float32),
                "sw2": np.asarray(shared_w2).astype(BF16),
            }
        )

    nc = _build_nc(C)
    trace = bool(int(os.environ.get("MOE_TRACE", "0")))
    res = run_bass_kernel_spmd(
        nc, in_maps, core_ids=list(range(NCORES)), trace=trace
    )
    LAST_EXEC_NS = res.exec_time_ns
    LAST_MEAN_EXEC_NS = res.mean_exec_time_ns
    LAST_TRACE = res.instructions_and_trace[1] if res.instructions_and_trace else None

    # ---- combine (un-shard) ----
    out = np.zeros((N, D), np.float32)
    for c in range(NCORES):
        out[c * NS : (c + 1) * NS] = np.asarray(res.results[c]["ysT"], np.float32).T
    for c in range(NCORES):
        yeT = res.results[c]["yeT"]
        for j in range(EPC):
            e = c * EPC + j
            idx = idx_per_e[e]
            out[idx] += np.asarray(yeT[j, :, : len(idx)], np.float32).T

    # host-side bias terms (zero in the shipped init; guarded for generality)
    shared_b2 = np.asarray(shared_b2, np.float32)
    if np.any(shared_b2):
        out += shared_b2.sum(0)[None, :]
    routed_b2 = np.asarray(routed_b2, np.float32)
    if np.any(routed_b2):
        w_full = np.where(sel, probs, 0.0).astype(np.float32)
        out += w_full @ routed_b2

    return out.reshape(B, T, D)


# revision 7
# speedup vs baseline: 1.1855x; 1.1129x over previous
"""DeepSeekMoE layer on 8 Trainium2 NeuronCores.

Strategy (expert-parallel, matching the sharding hint):
  - Host computes the (tiny) gate: softmax(x @ gate_w + gate_b), top-2
    routing, and per-expert token gather with capacity padding.  This is
    the control plane (<1% of FLOPs); all heavy matmuls run on device.
  - Each of the 8 cores owns E/8 = 4 routed experts: it receives the
    gathered tokens for those experts (transposed, bf16), the expert
    weights (bf16), and the per-(token,expert) combine weights.
  - The 2 shared experts are data-parallel over tokens: core c processes
    tokens [c*512, (c+1)*512), as two 256-token blocks that flow through
    the same pipeline as the routed blocks (with both shared experts
    accumulated in one K=2*FD matmul chain, and no combine weight).
  - Device per block: hT = w1^T @ xT (K=D in PSUM) -> Gelu(+b1) on ACT
    -> combine-weight multiply on DVE (hT stays FD-major) -> second
    matmul runs token-major: gT token-chunks are the stationary operand,
    w2 rows stream as the moving operand -> y[tok, D] staged row-wise in
    SBUF -> one contiguous DMA per 128-token chunk.
  - All inputs are pre-swizzled on the host into the exact SBUF tile
    layouts so every DMA is a fully contiguous copy; DMA issue is split
    across both HWDGE rings (sync + scalar engines).
  - Host scatter-adds routed expert outputs + shared outputs back into
    token order (each token appears in exactly 2 routed lists + 1 shared
    list, so a fp32 sum reproduces the reference combine).
All matmul inputs are bf16 (PSUM accumulates fp32); biases are applied
in fp32 (b1 via the ACT bias port; b2/gate_b host-side, and they are
zero-guarded so the common all-zero case costs nothing).
"""

import os
import sys
import types

import numpy as np

# ---------------------------------------------------------------------------
# Optional NTFF trace support under axon: concourse's trace path imports
# antenv.axon_hooks, which this image lacks; shim it with the boot helper.
# ---------------------------------------------------------------------------
def _install_trace_shim():
    try:
        if "antenv.axon_hooks" in sys.modules:
            return
        from trn_agent_boot.trn_boot import _ntff_profile_via_ctypes

        hook = _ntff_profile_via_ctypes("/opt/axon/libaxon_pjrt.so")
        mod = types.ModuleType("antenv.axon_hooks")
        mod.get_axon_ntff_profile_hook = lambda: hook
        mod.set_axon_ntff_profile_hook = lambda h: None
        sys.modules["antenv.axon_hooks"] = mod
    except Exception:
        pass


_install_trace_shim()

import ml_dtypes  # noqa: E402

import concourse.bacc as bacc  # noqa: E402
import concourse.mybir as mybir  # noqa: E402
import concourse.tile as tile  # noqa: E402
import concourse.bass_utils as bass_utils  # noqa: E402
from concourse.bass import ts, ds  # noqa: E402
from concourse.bass_utils import run_bass_kernel_spmd  # noqa: E402

try:  # zero-egress sandbox: skip artifact uploads during tracing
    bass_utils.upload_artifacts = lambda tmpdir: tmpdir
except Exception:
    pass

# Problem shapes (nn_DeepSeekMoE): x [B,T,D]; E routed experts (top-K),
# S shared experts, ffn dim FD.
B, T, D = 2, 2048, 5120
FD, E, S, K = 384, 32, 2, 2
N = B * T                     # 4096 tokens
P = 128
NCORES = 8
EPC = E // NCORES             # 4 routed experts per core
NS = N // NCORES              # 512 shared-expert tokens per core
NSH = NS // 2                 # shared-block token count (256)
KD = D // P                   # 40 K-subtiles over D
KF = FD // P                  # 3 K-subtiles over FD
NDC = D // 512                # 10 output-column chunks of 512 over D

BF16 = ml_dtypes.bfloat16

LAST_EXEC_NS = None
LAST_MEAN_EXEC_NS = None
LAST_TRACE = None


def _ceil_div(a, b):
    return -(-a // b)


def _build_nc(C: int, out_dt=mybir.dt.bfloat16):
    """Build the SPMD per-core Bass program. C = per-expert token capacity.

    Input DRAM layouts are pre-swizzled on the host so that every DMA below
    is a contiguous copy:
      xep  [EPC, P, KD, C]        x tokens for expert j, transposed, p-major
      w1p  [EPC, P, KD, FD]       routed w1, p-major over D
      w2p  [EPC, NDC, P, KF, 512] routed w2, p-major over FD, chunked over D
      xsp  [2, P, KD, NSH]        shared tokens, 2 blocks, transposed, p-major
      sw1p [S, P, KD, FD]         shared w1
      sw2p [S, NDC, P, KF, 512]   shared w2
      wgt  [EPC, P, C]            combine weights, broadcast over partitions
    Outputs are token-major: yE [EPC, C, D], ysE [NS, D].
    """
    f32 = mybir.dt.float32
    bf16 = mybir.dt.bfloat16
    nc = bacc.Bacc(None, target_bir_lowering=False)

    xep = nc.dram_tensor("xep", (EPC, P, KD, C), bf16, kind="ExternalInput")
    wgt = nc.dram_tensor("wgt", (EPC, P, C), f32, kind="ExternalInput")
    w1p = nc.dram_tensor("w1p", (EPC, P, KD, FD), bf16, kind="ExternalInput")
    rb1 = nc.dram_tensor("rb1", (EPC, FD), f32, kind="ExternalInput")
    w2p = nc.dram_tensor("w2p", (EPC, NDC, P, KF, 512), bf16, kind="ExternalInput")
    xsp = nc.dram_tensor("xsp", (2, P, KD, NSH), bf16, kind="ExternalInput")
    sw1p = nc.dram_tensor("sw1p", (S, P, KD, FD), bf16, kind="ExternalInput")
    sb1 = nc.dram_tensor("sb1", (S, FD), f32, kind="ExternalInput")
    sw2p = nc.dram_tensor("sw2p", (S, NDC, P, KF, 512), bf16, kind="ExternalInput")
    yE = nc.dram_tensor("yE", (EPC, C, D), out_dt, kind="ExternalOutput")
    ysE = nc.dram_tensor("ysE", (NS, D), out_dt, kind="ExternalOutput")

    gelu = mybir.ActivationFunctionType.Gelu
    KDH = KD // 2              # w1 / x stream in two K-halves

    with tile.TileContext(nc) as tc:
        with (
            tc.tile_pool(name="pw1", bufs=5) as pw1,
            tc.tile_pool(name="pxe", bufs=3) as pxe,
            tc.tile_pool(name="pw2", bufs=3) as pw2,
            tc.tile_pool(name="pg", bufs=4) as pg,
            tc.tile_pool(name="pwg", bufs=2) as pwg,
            tc.tile_pool(name="pyt", bufs=4) as pyt,
            tc.tile_pool(name="pb", bufs=1) as pb,
            tc.tile_pool(name="pps1", bufs=2, space="PSUM") as pps1,
            tc.tile_pool(name="pps2", bufs=4, space="PSUM") as pps2,
        ):
            # biases -> [P, n_subtiles] with the FD subtile index on free dim
            rb1_sb = pb.tile([P, EPC * KF], f32, tag="rb1")
            for e in range(EPC):
                nc.sync.dma_start(
                    rb1_sb[:, e * KF : (e + 1) * KF],
                    rb1[:][e].rearrange("(o p) -> p o", p=P),
                )
            sb1_sb = pb.tile([P, S * KF], f32, tag="sb1")
            for s in range(S):
                nc.sync.dma_start(
                    sb1_sb[:, s * KF : (s + 1) * KF],
                    sb1[:][s].rearrange("(o p) -> p o", p=P),
                )

            def load_khalves(pool, src_ap, width, tag, npieces=1):
                """Two [P, KD/2, width] tiles for a [P, KD, width] DRAM src."""
                tiles = []
                for h in range(2):
                    t = pool.tile([P, KDH, width], bf16, tag=tag, name="kh")
                    src = src_ap[:, h * KDH : (h + 1) * KDH]
                    step = KDH // npieces
                    for i in range(npieces):
                        nc.sync.dma_start(
                            t[:, i * step : (i + 1) * step],
                            src[:, i * step : (i + 1) * step],
                        )
                    tiles.append(t)
                return tiles

            def process_block(x_tiles, ntok, w1_list, boffs, wgt_tile,
                              w2_list, out_ap):
                """One token block through both FFN matmuls.

                x_tiles: two [P, KDH, ntok] tiles (K-halves of x^T)
                w1_list: per source, two [P, KDH, FD] tiles
                boffs:   per source, bias column offset into its bias tile
                w2_list: per source, [NDC, P, KF, 512] DRAM AP
                out_ap:  [ntok, D] DRAM AP
                """
                nsrc = len(w1_list)
                g_tiles = []
                for si in range(nsrc):
                    bias_sb, boff = boffs[si]
                    g_t = pg.tile([P, KF, ntok], bf16, tag="g", name="g_t")
                    for mi in range(KF):
                        ph = pps1.tile([P, 512], f32, tag="ph", name="ph")[:, :ntok]
                        for kd in range(KD):
                            w1h = w1_list[si][kd // KDH]
                            xh = x_tiles[kd // KDH]
                            nc.tensor.matmul(
                                ph,
                                w1h[:, kd % KDH, ts(mi, P)],
                                xh[:, kd % KDH, :],
                                start=(kd == 0),
                                stop=(kd == KD - 1),
                            )
                        nc.scalar.activation(
                            g_t[:, mi, :],
                            ph,
                            gelu,
                            bias=bias_sb[:, boff + mi : boff + mi + 1],
                        )
                        if wgt_tile is not None:
                            nc.vector.tensor_mul(
                                out=g_t[:, mi, :],
                                in0=g_t[:, mi, :],
                                in1=wgt_tile,
                            )
                    g_tiles.append(g_t)

                # token-major second matmul: gT chunks stationary, w2 moving
                nchunk = _ceil_div(ntok, P)
                nmm = nsrc * KF
                yrows = [
                    pyt.tile([P, D], out_dt, tag="yrow", name="yrow")
                    for _ in range(nchunk)
                ]
                for mdc in range(NDC):
                    w2ts = []
                    for si in range(nsrc):
                        w2t = pw2.tile([P, KF, 512], bf16, tag="w2", name="w2t")
                        nc.scalar.dma_start(w2t, w2_list[si][mdc])
                        w2ts.append(w2t)
                    for ci in range(nchunk):
                        cw = min(P, ntok - ci * P)
                        py = pps2.tile([P, 512], f32, tag="py", name="py")[:cw]
                        imm = 0
                        for si in range(nsrc):
                            for kf in range(KF):
                                nc.tensor.matmul(
                                    py,
                                    g_tiles[si][:, kf, ds(ci * P, cw)],
                                    w2ts[si][:, kf, :],
                                    start=(imm == 0),
                                    stop=(imm == nmm - 1),
                                )
                                imm += 1
                        nc.vector.tensor_copy(
                            out=yrows[ci][:cw, ts(mdc, 512)], in_=py
                        )
                for ci in range(nchunk):
                    cw = min(P, ntok - ci * P)
                    nc.scalar.dma_start(out_ap[ds(ci * P, cw), :], yrows[ci][:cw])

            # ---------------- routed experts ----------------
            for e in range(EPC):
                npieces = 2 if e == 0 else 1
                x_tiles = load_khalves(pxe, xep[:][e], C, "xe", npieces)
                wgt_t = pwg.tile([P, C], f32, tag="wg", name="wgt_t")
                nc.sync.dma_start(wgt_t, wgt[:][e])
                w1_t = load_khalves(pw1, w1p[:][e], FD, "w1", npieces)
                process_block(
                    x_tiles, C, [w1_t], [(rb1_sb, e * KF)], wgt_t,
                    [w2p[:][e]], yE[:][e],
                )

            # ---------------- shared experts (token-parallel) ----------------
            sw1_t = [load_khalves(pw1, sw1p[:][s], FD, "w1") for s in range(S)]
            for h in range(2):
                xs_tiles = load_khalves(pxe, xsp[:][h], NSH, "xe")
                process_block(
                    xs_tiles, NSH, sw1_t,
                    [(sb1_sb, s * KF) for s in range(S)], None,
                    [sw2p[:][s] for s in range(S)],
                    ysE[:][ds(h * NSH, NSH), :],
                )

    nc.compile()
    return nc


def kernel(x, shared_w1, shared_b1, shared_w2, shared_b2,
           routed_w1, routed_b1, routed_w2, routed_b2, gate_w, gate_b):
    global LAST_EXEC_NS, LAST_MEAN_EXEC_NS, LAST_TRACE

    x = np.asarray(x, np.float32)
    x2d = np.ascontiguousarray(x.reshape(N, D))

    # ---- gating / routing (control plane) ----
    logits = x2d @ np.asarray(gate_w, np.float32) + np.asarray(gate_b, np.float32)
    logits -= logits.max(axis=-1, keepdims=True)
    probs = np.exp(logits)
    probs /= probs.sum(axis=-1, keepdims=True)                  # [N, E]
    top2 = np.argpartition(-probs, K - 1, axis=-1)[:, :K]       # [N, K]
    sel = np.zeros((N, E), np.bool_)
    sel[np.arange(N)[:, None], top2] = True

    idx_per_e = [np.nonzero(sel[:, e])[0] for e in range(E)]
    counts = np.array([len(i) for i in idx_per_e])
    C = max(128, int(np.ceil(counts.max() / 32)) * 32)

    x_bf = x2d.astype(BF16)

    def pmajor_T(rows):
        """[n, D] fp32/bf16 rows -> x^T p-major [P, KD, n]."""
        return rows.T.reshape(KD, P, rows.shape[0]).transpose(1, 0, 2)

    # pre-swizzled shared weights (identical on every core)
    sw1p = np.ascontiguousarray(
        np.asarray(shared_w1).astype(BF16).reshape(S, KD, P, FD).transpose(0, 2, 1, 3)
    )
    sw2p = np.ascontiguousarray(
        np.asarray(shared_w2).astype(BF16)
        .reshape(S, KF, P, NDC, 512).transpose(0, 3, 2, 1, 4)
    )
    rw1_bf = np.asarray(routed_w1).astype(BF16)
    rw2_bf = np.asarray(routed_w2).astype(BF16)
    rb1_f = np.asarray(routed_b1, np.float32)
    sb1_f = np.asarray(shared_b1, np.float32)

    in_maps = []
    for c in range(NCORES):
        xep = np.zeros((EPC, P, KD, C), BF16)
        wgtb = np.zeros((EPC, P, C), np.float32)
        for j in range(EPC):
            e = c * EPC + j
            idx = idx_per_e[e]
            xep[j, :, :, : len(idx)] = pmajor_T(x_bf[idx])
            wgtb[j, :, : len(idx)] = probs[idx, e][None, :]
        xsp = np.stack(
            [
                pmajor_T(x_bf[c * NS + h * NSH : c * NS + (h + 1) * NSH])
                for h in range(2)
            ]
        )
        w1c = rw1_bf[c * EPC : (c + 1) * EPC]
        w2c = rw2_bf[c * EPC : (c + 1) * EPC]
        in_maps.append(
            {
                "xep": xep,
                "wgt": wgtb,
                "w1p": np.ascontiguousarray(
                    w1c.reshape(EPC, KD, P, FD).transpose(0, 2, 1, 3)
                ),
                "rb1": np.ascontiguousarray(rb1_f[c * EPC : (c + 1) * EPC]),
                "w2p": np.ascontiguousarray(
                    w2c.reshape(EPC, KF, P, NDC, 512).transpose(0, 3, 2, 1, 4)
                ),
                "xsp": np.ascontiguousarray(xsp),
                "sw1p": sw1p,
                "sb1": sb1_f,
                "sw2p": sw2p,
            }
        )

    nc = _build_nc(C)
    trace = bool(int(os.environ.get("MOE_TRACE", "0")))
    res = run_bass_kernel_spmd(
        nc, in_maps, core_ids=list(range(NCORES)), trace=trace
    )
    LAST_EXEC_NS = res.exec_time_ns
    LAST_MEAN_EXEC_NS = res.mean_exec_time_ns
    LAST_TRACE = res.instructions_and_trace[1] if res.instructions_and_trace else None

    # ---- combine (un-shard) ----
    out = np.zeros((N, D), np.float32)
    for c in range(NCORES):
        out[c * NS : (c + 1) * NS] = np.asarray(res.results[c]["ysE"], np.float32)
    for c in range(NCORES):
        yEc = res.results[c]["yE"]
        for j in range(EPC):
            e = c * EPC + j
            idx = idx_per_e[e]
            out[idx] += np.asarray(yEc[j, : len(idx), :], np.float32)

    # host-side bias terms (zero in the shipped init; guarded for generality)
    shared_b2 = np.asarray(shared_b2, np.float32)
    if np.any(shared_b2):
        out += shared_b2.sum(0)[None, :]
    routed_b2 = np.asarray(routed_b2, np.float32)
    if np.any(routed_b2):
        w_full = np.where(sel, probs, 0.0).astype(np.float32)
        out += w_full @ routed_b2

    return out.reshape(B, T, D)
